# revision 25
# baseline (speedup 1.0000x reference)
"""Distributed causal multi-head attention (B=2, S=2048, H=2048, 16 heads) on
8 TRN2 NeuronCores.

Sharding: core c handles batch b = c // 4 and the 4-head group g = c % 4
(heads 4g..4g+3). Each core computes its heads' QKV projection, causal
attention, and the partial output projection against its 512 columns of
w_proj (Megatron row-parallel). No on-device collectives: the 4 partial
outputs per batch are summed on the host during unsharding.

Schedule notes (v4 — startup/stall/tail surgery over the v0 schedule):
- The kernel is PE-column-throughput-bound: ~658K moving columns of bf16
  matmul. All scheduling aims to keep TensorE streaming continuously.
  (fp8/DoubleRow was evaluated and rejected: one e4m3-quantized GEMM
  stage injects 3-5e-2 relative error end-to-end vs the 2e-2 gate.)
- Startup: x tiles (halved/quartered) and the per-tile wq01 weight
  slices are DMAed in exact pass-1 consumption order, striped per
  contraction tile h across the sync/gpsimd/scalar queues, issued
  before everything else: a single queue only sustains ~120-180 GB/s,
  so in-order delivery needs all three. The first matmul starts after
  ~200KB instead of ~10MB and the PE never starves during the x load.
- All weights are tile-row-major in DRAM ([ntiles*128, cols]) so every
  per-tile slice DMA is one contiguous 64-512KB read; wk01/wq23/wk23/
  wv/wp stream on sync+gpsimd right behind x in first-use order (wk01
  slice 0 lands ~40us, needed ~44us).
- Per-head slot: diagonal singles run FIRST (their exp -> gpsimd-mask ->
  PV-pop chain spans three engines; issuing them early hides the
  latency under the dense full pairs). From the 3rd score pair on, one
  independent filler chunk (block j-1 out-projection / block j+1
  v-projection, 4-8 matmuls) is interleaved per pair: the PE outruns
  exp by ~85ns/pair and would otherwise stall on the 2-buffer scores
  PSUM ping-pong. One filler chunk is held back for the final PV drain.
- The denominator chain (ones-matmul, reciprocal, 1/l multiply) is
  emitted before the remaining fillers so recip+renorm sit at the front
  of the DVE queue: the next slots' first PV matmul has a PSUM WAR
  dependency on the renorm (ypp ping-pong) and previously stalled.
- Out-projection PSUM drains split across DVE (o=0,1) and Scalar
  (o=2,3); v drains on Scalar; diagonal-mask multiplies on GpSimd
  (GPSIMD cannot read PSUM, so it only gets SBUF-to-SBUF work).
- Tail: the last block's out-projection runs in its own PSUM pool scope
  (4 accumulator banks, no drain-gating) with per-chunk stores fanned
  across sync/gpsimd/scalar so only ~128KB is in flight at run end.
  The remaining ~14us (host doorbell ~5us, engine preamble ~3.5us,
  ~250-semaphore clear epilogue ~8us) is framework-fixed.

Device compute is bf16 on the TensorEngine with fp32 PSUM accumulation;
softmax runs in fp32. Scores are bounded (~±5 post-scale), so exp needs
no max-subtraction. Attention scores are computed directly in transposed
orientation (scoresT[k_pos, q_pos] via lhsT=k-tile, rhs=q-block), which
is what the PV matmul wants as its moving operand. The softmax
denominator comes from a ones matmul over a DVE running sum of probs
tiles, and 1/l is applied when draining the PV accumulator.
"""
import sys

for _p in ("/opt/trn_rl_repo", "/opt/pypackages"):
    if _p not in sys.path:
        sys.path.append(_p)

import numpy as np
from ml_dtypes import bfloat16

import concourse.bass as bass  # noqa: F401
import concourse.mybir as mybir
import concourse.tile as tile
import concourse.bacc as bacc
from concourse.bass_utils import run_bass_kernel_spmd


def _ensure_ntff_hook():
    """bass_utils imports antenv.axon_hooks when trace is requested (e.g. via
    BASS_TRACE=1); this image's antenv lacks that module. Provide it, wired to
    the axon .so when available, so tracing works and never crashes."""
    import types
    name = "antenv.axon_hooks"
    if name in sys.modules:
        return
    mod = types.ModuleType(name)
    mod._hook = None
    mod.set_axon_ntff_profile_hook = lambda h: setattr(mod, "_hook", h)
    mod.get_axon_ntff_profile_hook = lambda: mod._hook
    sys.modules[name] = mod
    try:
        import antenv
        antenv.axon_hooks = mod
        from trn_agent_boot.trn_boot import _ntff_profile_via_ctypes
        mod._hook = _ntff_profile_via_ctypes("/opt/axon/libaxon_pjrt.so")
    except Exception:
        pass


_ensure_ntff_hook()

B, S, H = 2, 2048, 2048
NH = 16
HD = 128
NHL = 4                      # heads per core
HL = NHL * HD                # 512 local head-dims
N_CORES = 8
F32 = mybir.dt.float32
BF16 = mybir.dt.bfloat16
SCALE = float(1.0 / np.sqrt(HD))

NHT = H // 128               # 16 contraction tiles
NQ = S // 128                # 16 query sub-tiles
NBLK = 4                     # 512-wide query blocks

LAST_RESULT = None
_CACHED_NC = None


def build_graph():
    import random
    random.seed(0)
    np.random.seed(0)
    nc = bacc.Bacc("TRN2", target_bir_lowering=False, num_devices=N_CORES)
    # x stays in [H, S] layout: per-tile DMAs read DRAM linearly.
    xT_d = nc.declare_dram_parameter("xT", [H, S], BF16, isOutput=False)
    # All weights are tile-ROW-major ([ntiles*128, cols]) so per-tile
    # slices are contiguous 64-512KB DRAM reads; they stream on the two
    # input queues in exact consumption order behind x.
    wq01_d = nc.declare_dram_parameter("wq01", [NHT * 128, 256], BF16,
                                       isOutput=False)
    wk01_d = nc.declare_dram_parameter("wk01", [NHT * 128, 256], BF16,
                                       isOutput=False)
    wq23_d = nc.declare_dram_parameter("wq23", [NHT * 128, 256], BF16,
                                       isOutput=False)
    wk23_d = nc.declare_dram_parameter("wk23", [NHT * 128, 256], BF16,
                                       isOutput=False)
    wv_d = nc.declare_dram_parameter("wvP", [NHT * 128, HL], BF16,
                                     isOutput=False)
    wp_d = nc.declare_dram_parameter("wpP", [NHL * 128, H], BF16,
                                     isOutput=False)
    onesb_d = nc.declare_dram_parameter("onesb", [128, 128], BF16,
                                        isOutput=False)
    # upper-triangular (incl diagonal) 0/1 keep-mask for post-exp zeroing
    m01_d = nc.declare_dram_parameter("m01", [128, 128], BF16, isOutput=False)
    out_d = nc.declare_dram_parameter("out", [S, H], BF16, isOutput=True)

    Copy = mybir.ActivationFunctionType.Copy
    Exp = mybir.ActivationFunctionType.Exp

    with tile.TileContext(nc) as tc:
        with tc.tile_pool(name="persist", bufs=1) as pp:
            # ---- persistent SBUF tiles ----
            xT = pp.tile([128, NHT * S], BF16, tag="x", name="xT")
            wq01 = pp.tile([128, NHT * 256], BF16, tag="wq01", name="wq01")
            wk01 = pp.tile([128, NHT * 256], BF16, tag="wk01", name="wk01")
            wq23 = pp.tile([128, NHT * 256], BF16, tag="wq23", name="wq23")
            wk23 = pp.tile([128, NHT * 256], BF16, tag="wk23", name="wk23")
            wv = pp.tile([128, NHT * HL], BF16, tag="wv", name="wv")
            wp = pp.tile([128, NHL * H], BF16, tag="wp", name="wp")
            qT = [pp.tile([128, S], BF16, tag=f"q{h}", name=f"qT{h}")
                  for h in range(NHL)]
            kT = [pp.tile([128, S], BF16, tag=f"k{h}", name=f"kT{h}")
                  for h in range(NHL)]
            v_sb = [pp.tile([128, HL], BF16, tag=f"v{t}", name=f"v{t}")
                    for t in range(NQ)]
            # yT double-buffered by block parity (block j in parity j%2)
            yT = [[pp.tile([128, 512], BF16, tag=f"y{h}_{par}",
                           name=f"y{h}_{par}") for par in range(2)]
                  for h in range(NHL)]
            onesb_sb = pp.tile([128, 128], BF16, tag="onesb", name="onesb_sb")
            m01_sb = pp.tile([128, 128], BF16, tag="m01", name="m01_sb")

            # ---- startup DMAs in pass-1 consumption order ----
            # tile h's weight slice + two x halves, alternating queues so
            # aggregate delivery stays in order and ahead of the PE.
            for h in range(NHT):
                eng = (nc.sync, nc.gpsimd, nc.scalar)[h % 3]
                eng.dma_start(wq01[:, 256 * h:256 * (h + 1)],
                              wq01_d[128 * h:128 * (h + 1), :])
                # finer chunks for the first tiles so the first matmul can
                # start as early as possible
                nx = 4 if h < 2 else 2
                cw = S // nx
                for ci in range(nx):
                    eng.dma_start(xT[:, S * h + cw * ci:S * h + cw * (ci + 1)],
                                  xT_d[128 * h:128 * (h + 1),
                                       cw * ci:cw * (ci + 1)])
            # Remaining weights stream per-tile on the same two queues right
            # behind x, in consumption order: wk01 slice h lands ~40us,
            # needed ~44+2h us (it previously arrived after pass 2 began).
            # A third queue would steal HBM bandwidth from the x stream.
            for wtile, wdram, cols in ((wk01, wk01_d, 256),
                                       (wq23, wq23_d, 256),
                                       (wk23, wk23_d, 256),
                                       (wv, wv_d, HL)):
                for h in range(NHT):
                    eng = nc.sync if h % 2 == 0 else nc.gpsimd
                    eng.dma_start(wtile[:, cols * h:cols * (h + 1)],
                                  wdram[128 * h:128 * (h + 1), :])
            for k in range(NHL):
                eng = nc.sync if k % 2 == 0 else nc.gpsimd
                eng.dma_start(wp[:, H * k:H * (k + 1)],
                              wp_d[128 * k:128 * (k + 1), :])
            nc.scalar.dma_start(onesb_sb[:], onesb_d[:])
            nc.scalar.dma_start(m01_sb[:], m01_d[:])

            # ---------------- Phase B: Q/K projections ----------------
            # Four passes (wq-hp01, wk-hp01, wq-hp23, wk-hp23); each pass
            # accumulates 8 PSUM tiles (2 heads x 4 token chunks) over the
            # 16 contraction tiles.  PSUM drains alternate Scalar/Vector.
            with tc.tile_pool(name="pB", bufs=8, space="PSUM") as pB:
                def proj_pass(wtile, dsts):
                    pss = [pB.tile([128, 512], F32, tag="pb", name="pb")
                           for _ in range(8)]
                    for h in range(NHT):
                        for gi in range(2):
                            lhsT = wtile[:, 256 * h + 128 * gi:
                                         256 * h + 128 * (gi + 1)]
                            for s in range(4):
                                nc.tensor.matmul(
                                    pss[4 * gi + s][:], lhsT,
                                    xT[:, S * h + 512 * s:
                                       S * h + 512 * (s + 1)],
                                    start=(h == 0), stop=(h == NHT - 1))
                    for gi in range(2):
                        for s in range(4):
                            dst = dsts[gi][:, 512 * s:512 * (s + 1)]
                            src = pss[4 * gi + s][:]
                            if s % 2 == 0:
                                nc.scalar.activation(dst, src, Copy)
                            else:
                                nc.vector.tensor_copy(dst, src)

                proj_pass(wq01, (qT[0], qT[1]))
                proj_pass(wk01, (kT[0], kT[1]))
                proj_pass(wq23, (qT[2], qT[3]))
                proj_pass(wk23, (kT[2], kT[3]))

            # ---------- Phase C: v-proj + attention + out-proj ----------
            with (
                tc.tile_pool(name="spp", bufs=2, space="PSUM") as spp,
                tc.tile_pool(name="ypp", bufs=2, space="PSUM") as ypp,
                tc.tile_pool(name="accp", bufs=2, space="PSUM") as accp,
                tc.tile_pool(name="ptp", bufs=5) as ptp,
                tc.tile_pool(name="bcp", bufs=1) as bcp,
                tc.tile_pool(name="otp", bufs=2) as otp,
                tc.tile_pool(name="lsp", bufs=2) as lsp,
            ):
                def vproj_mms(st, acc, h2s):
                    for h2 in h2s:
                        nc.tensor.matmul(
                            acc[:],
                            xT[:, S * h2 + 128 * st:S * h2 + 128 * (st + 1)],
                            wv[:, 512 * h2:512 * (h2 + 1)],
                            start=(h2 == 0), stop=(h2 == NHT - 1))

                def vproj_tile(st):
                    acc = accp.tile([128, 512], F32, tag="acc", name="vm")
                    vproj_mms(st, acc, range(NHT))
                    nc.scalar.activation(v_sb[st][:], acc[:], Copy)

                def oproj_chunk(q, o, ot):
                    jq, q4 = divmod(q, 4)
                    acc = accp.tile([128, 512], F32, tag="acc", name="po")
                    for k in range(4):
                        nc.tensor.matmul(
                            acc[:],
                            yT[k][jq % 2][:, 128 * q4:128 * (q4 + 1)],
                            wp[:, 2048 * k + 512 * o:
                               2048 * k + 512 * (o + 1)],
                            start=(k == 0), stop=(k == 3))
                    dst = ot[:, 512 * o:512 * (o + 1)]
                    # split drains across DVE and Scalar: DVE also carries
                    # the softmax-denominator chain and was the slot-end
                    # bottleneck
                    if o < 2:
                        nc.vector.tensor_copy(dst, acc[:])
                    else:
                        nc.scalar.activation(dst, acc[:], Copy)

                # block 0's own v-projection (token tiles 0..3)
                for i in range(4):
                    vproj_tile(i)

                for j in range(NBLK):
                    nt = 4 * (j + 1)
                    for h in range(NHL):
                        # All blocks: full k-tile PAIRS share one 2-bank
                        # PSUM tile so a single exp instruction covers 1024
                        # columns.  The softmax denominator is a bf16
                        # running sum of probs tiles on DVE, finished by ONE
                        # ones-matmul per head; masked diagonal probs are
                        # zeroed by a 0/1 mul on GpSimd.
                        yp = ypp.tile([128, 512], F32, tag="yt", name="yp")
                        lsum = lsp.tile([128, 512], BF16, tag="lsum",
                                        name="lsum")
                        pend = []
                        first_ls = True
                        npops = [0]

                        # Independent PE filler work for this slot, emitted
                        # one chunk at a time between score pairs and around
                        # the final PV drain: block j-1's out-projection and
                        # block j+1's v-projection.
                        fillers = []
                        if j > 0:
                            q = 4 * (j - 1) + h
                            ot = otp.tile([128, H], BF16, tag="ot", name="ot")

                            def mk_oproj(o, q=q, ot=ot):
                                def f():
                                    oproj_chunk(q, o, ot)
                                    if o == 3:
                                        # rotate store queues so the last
                                        # block's stores don't back up sync
                                        # right before the tail flush
                                        seng = (nc.sync, nc.scalar,
                                                nc.gpsimd)[q % 3]
                                        seng.dma_start(
                                            out_d[128 * q:128 * (q + 1), :],
                                            ot[:])
                                return f

                            for o in range(4):
                                fillers.append(mk_oproj(o))
                        if j < 3:
                            st = 4 * (j + 1) + h
                            vst = {}

                            def vh0(st=st, vst=vst):
                                vst["acc"] = accp.tile([128, 512], F32,
                                                       tag="acc", name="vm")
                                vproj_mms(st, vst["acc"], range(0, 8))

                            def vh1(st=st, vst=vst):
                                vproj_mms(st, vst["acc"], range(8, NHT))
                                nc.scalar.activation(v_sb[st][:],
                                                     vst["acc"][:], Copy)

                            fillers.append(vh0)
                            fillers.append(vh1)

                        def pv_pop():
                            t, tile_, base, r0 = pend.pop(0)
                            nc.tensor.matmul(
                                yp[:, r0:512],
                                v_sb[t][:, 128 * h:128 * (h + 1)],
                                tile_[:, base + r0:base + 512],
                                start=(npops[0] == 0),
                                stop=(npops[0] == nt - 1))
                            npops[0] += 1

                        # Diagonal singles FIRST: their exp -> mask -> pop
                        # chains span three engines, so issuing them early
                        # hides the latency under the dense full pairs, and
                        # the end-of-head drain only waits on short chains.
                        for t in range(4 * j, nt):
                            r0 = 0 if t <= 4 * j else 128 * (t - 4 * j)
                            spw = spp.tile([128, 1024], F32, tag="sc",
                                           name="sp")
                            ptw = ptp.tile([128, 1024], BF16, tag="pt",
                                           name="pt")
                            nc.tensor.matmul(
                                spw[:, r0:512],
                                kT[h][:, 128 * t:128 * (t + 1)],
                                qT[h][:, 512 * j + r0:512 * (j + 1)],
                                start=True, stop=True)
                            nc.scalar.activation(
                                ptw[:, r0:512], spw[:, r0:512], Exp,
                                scale=SCALE)
                            # zero the strictly-lower (masked) part of the
                            # diagonal 128-block
                            nc.gpsimd.tensor_mul(
                                ptw[:, r0:r0 + 128],
                                ptw[:, r0:r0 + 128], m01_sb[:])
                            if first_ls:
                                nc.vector.tensor_copy(lsum[:],
                                                      ptw[:, 0:512])
                                first_ls = False
                            else:
                                nc.vector.tensor_add(
                                    lsum[:, r0:512], lsum[:, r0:512],
                                    ptw[:, r0:512])
                            pend.append((t, ptw, 0, r0))
                            while len(pend) > 5:
                                pv_pop()
                        # full k-tile pairs; one 1024-col exp per pair.
                        # From the 3rd pair on, interleave one filler chunk
                        # per pair (keeping one in reserve for the final
                        # drain): the PE outruns exp by ~85ns/pair and would
                        # otherwise stall on the scores-PSUM ping-pong.
                        for idx, t in enumerate(range(0, 4 * j, 2)):
                            spw = spp.tile([128, 1024], F32, tag="sc",
                                           name="sp")
                            ptw = ptp.tile([128, 1024], BF16, tag="pt",
                                           name="pt")
                            for half in range(2):
                                nc.tensor.matmul(
                                    spw[:, 512 * half:512 * (half + 1)],
                                    kT[h][:, 128 * (t + half):
                                          128 * (t + half + 1)],
                                    qT[h][:, 512 * j:512 * (j + 1)],
                                    start=True, stop=True)
                            nc.scalar.activation(
                                ptw[:], spw[:], Exp, scale=SCALE)
                            nc.vector.tensor_add(lsum[:], lsum[:],
                                                 ptw[:, 0:512])
                            nc.vector.tensor_add(lsum[:], lsum[:],
                                                 ptw[:, 512:1024])
                            pend.append((t, ptw, 0, 0))
                            pend.append((t + 1, ptw, 512, 0))
                            while len(pend) > 5:
                                pv_pop()
                            if idx >= 2 and len(fillers) > 1:
                                fillers.pop(0)()

                        # one filler chunk while the last exp/mask ops drain
                        if fillers:
                            fillers.pop(0)()
                        while pend:
                            pv_pop()
                        # denominator chain immediately: recip + yT-mul go
                        # to the front of the DVE queue so the next slots'
                        # PV accumulator (PSUM WAR on yp) never waits
                        lp = spp.tile([128, 1024], F32, tag="sc", name="lp2")
                        nc.tensor.matmul(lp[:, 0:512], onesb_sb[:], lsum[:],
                                         start=True, stop=True)
                        bcs = bcp.tile([128, 512], F32, tag="bcs", name="bcs")
                        nc.vector.reciprocal_approx_fast(bcs[:], lp[:, 0:512])
                        nc.vector.tensor_mul(yT[h][j % 2][:], yp[:], bcs[:])
                        while fillers:
                            fillers.pop(0)()

            # tail: out-projection for the last block in its own PSUM pool
            # scope — the phase-C pools are closed, so four accumulator
            # banks are free and a chunk's drain never gates the next
            # chunk's matmuls
            with (
                tc.tile_pool(name="taccp", bufs=4, space="PSUM") as taccp,
                tc.tile_pool(name="totp", bufs=2) as totp,
            ):
                for q4 in range(4):
                    q = 12 + q4
                    ot = totp.tile([128, H], BF16, tag="ot", name="ott")
                    for o in range(4):
                        acc = taccp.tile([128, 512], F32, tag="acc",
                                         name="pot")
                        for k in range(4):
                            nc.tensor.matmul(
                                acc[:],
                                yT[k][1][:, 128 * q4:128 * (q4 + 1)],
                                wp[:, 2048 * k + 512 * o:
                                   2048 * k + 512 * (o + 1)],
                                start=(k == 0), stop=(k == 3))
                        dst = ot[:, 512 * o:512 * (o + 1)]
                        if o % 2 == 1:
                            nc.scalar.activation(dst, acc[:], Copy)
                        else:
                            nc.vector.tensor_copy(dst, acc[:])
                        # store per chunk, fanned across queues, so only the
                        # last 512 columns remain in flight at run end
                        eng = (nc.gpsimd, nc.scalar, nc.gpsimd, nc.sync)[o]
                        eng.dma_start(
                            out_d[128 * q:128 * (q + 1),
                                  512 * o:512 * (o + 1)], dst)

    nc.compile()
    return nc


def _get_nc():
    global _CACHED_NC
    if _CACHED_NC is None:
        _CACHED_NC = build_graph()
    return _CACHED_NC


def kernel(x, w_attn, w_proj):
    global LAST_RESULT
    nc = _get_nc()
    onesb = np.ones((128, 128), bfloat16)
    m01 = np.triu(np.ones((128, 128), np.float32)).astype(bfloat16)
    in_maps = []
    for c in range(N_CORES):
        b, g = divmod(c, 4)
        lo, hi = HL * g, HL * (g + 1)
        xTb = np.ascontiguousarray(x[b].T).astype(bfloat16)
        wqT = np.ascontiguousarray(w_attn[lo:hi, :].T).astype(bfloat16)
        wkT = np.ascontiguousarray(w_attn[H + lo:H + hi, :].T).astype(bfloat16)
        wvT = np.ascontiguousarray(w_attn[2 * H + lo:2 * H + hi, :].T
                                   ).astype(bfloat16)
        wpT = np.ascontiguousarray(w_proj[:, lo:hi].T).astype(bfloat16)
        in_maps.append({
            "xT": xTb,
            # weights tile-row-major: rows 128h..128h+128 = contraction
            # tile h (contiguous DRAM block per slice DMA)
            "wq01": np.ascontiguousarray(wqT[:, 0:256]),
            "wk01": np.ascontiguousarray(wkT[:, 0:256]),
            "wq23": np.ascontiguousarray(wqT[:, 256:512]),
            "wk23": np.ascontiguousarray(wkT[:, 256:512]),
            "wvP": wvT,
            "wpP": wpT,
            "onesb": onesb,
            "m01": m01,
        })
    res = run_bass_kernel_spmd(nc, in_maps, core_ids=list(range(N_CORES)))
    LAST_RESULT = res
    outs = [np.asarray(res.results[c]["out"], dtype=np.float32)
            for c in range(N_CORES)]
    out = np.empty((B, S, H), np.float32)
    out[0] = outs[0] + outs[1] + outs[2] + outs[3]
    out[1] = outs[4] + outs[5] + outs[6] + outs[7]
    return out


# revision 26
# speedup vs baseline: 1.0075x; 1.0075x over previous
"""Distributed causal multi-head attention (B=2, S=2048, H=2048, 16 heads) on
8 TRN2 NeuronCores.

Sharding: core c handles batch b = c // 4 and the 4-head group g = c % 4
(heads 4g..4g+3). Each core computes its heads' QKV projection, causal
attention, and the partial output projection against its 512 columns of
w_proj (Megatron row-parallel). No on-device collectives: the 4 partial
outputs per batch are summed on the host during unsharding.

Schedule notes (v4 — startup/stall/tail surgery over the v0 schedule):
- The kernel is PE-column-throughput-bound: ~658K moving columns of bf16
  matmul. All scheduling aims to keep TensorE streaming continuously.
  (fp8/DoubleRow was evaluated and rejected: one e4m3-quantized GEMM
  stage injects 3-5e-2 relative error end-to-end vs the 2e-2 gate.)
- Startup: x tiles (halved/quartered) and the per-tile wq01 weight
  slices are DMAed in exact pass-1 consumption order, striped per
  contraction tile h across the sync/gpsimd/scalar queues, issued
  before everything else: a single queue only sustains ~120-180 GB/s,
  so in-order delivery needs all three. The first matmul starts after
  ~200KB instead of ~10MB and the PE never starves during the x load.
- All weights are tile-row-major in DRAM ([ntiles*128, cols]) so every
  per-tile slice DMA is one contiguous 64-512KB read; wk01/wq23/wk23/
  wv/wp stream on sync+gpsimd right behind x in first-use order (wk01
  slice 0 lands ~40us, needed ~44us).
- Per-head slot: diagonal singles run FIRST (their exp -> gpsimd-mask ->
  PV-pop chain spans three engines; issuing them early hides the
  latency under the dense full pairs). From the 3rd score pair on, one
  independent filler chunk (block j-1 out-projection / block j+1
  v-projection, 4-8 matmuls) is interleaved per pair: the PE outruns
  exp by ~85ns/pair and would otherwise stall on the 2-buffer scores
  PSUM ping-pong. One filler chunk is held back for the final PV drain.
- The denominator chain (ones-matmul, reciprocal, 1/l multiply) is
  emitted before the remaining fillers so recip+renorm sit at the front
  of the DVE queue: the next slots' first PV matmul has a PSUM WAR
  dependency on the renorm (ypp ping-pong) and previously stalled.
- Out-projection PSUM drains split across DVE (o=0,1) and Scalar
  (o=2,3); v drains on Scalar; diagonal-mask multiplies on GpSimd
  (GPSIMD cannot read PSUM, so it only gets SBUF-to-SBUF work).
- Tail: the last block's out-projection runs in its own PSUM pool scope
  (4 accumulator banks, no drain-gating) with per-chunk stores fanned
  across sync/gpsimd/scalar so only ~128KB is in flight at run end.
  The remaining ~14us (host doorbell ~5us, engine preamble ~3.5us,
  ~250-semaphore clear epilogue ~8us) is framework-fixed.

Device compute is bf16 on the TensorEngine with fp32 PSUM accumulation;
softmax runs in fp32. Scores are bounded (~±5 post-scale), so exp needs
no max-subtraction. Attention scores are computed directly in transposed
orientation (scoresT[k_pos, q_pos] via lhsT=k-tile, rhs=q-block), which
is what the PV matmul wants as its moving operand. The softmax
denominator comes from a ones matmul over a DVE running sum of probs
tiles, and 1/l is applied when draining the PV accumulator.
"""
import sys

for _p in ("/opt/trn_rl_repo", "/opt/pypackages"):
    if _p not in sys.path:
        sys.path.append(_p)

import numpy as np
from ml_dtypes import bfloat16

import concourse.bass as bass  # noqa: F401
import concourse.mybir as mybir
import concourse.tile as tile
import concourse.bacc as bacc
from concourse.bass_utils import run_bass_kernel_spmd


def _ensure_ntff_hook():
    """bass_utils imports antenv.axon_hooks when trace is requested (e.g. via
    BASS_TRACE=1); this image's antenv lacks that module. Provide it, wired to
    the axon .so when available, so tracing works and never crashes."""
    import types
    name = "antenv.axon_hooks"
    if name in sys.modules:
        return
    mod = types.ModuleType(name)
    mod._hook = None
    mod.set_axon_ntff_profile_hook = lambda h: setattr(mod, "_hook", h)
    mod.get_axon_ntff_profile_hook = lambda: mod._hook
    sys.modules[name] = mod
    try:
        import antenv
        antenv.axon_hooks = mod
        from trn_agent_boot.trn_boot import _ntff_profile_via_ctypes
        mod._hook = _ntff_profile_via_ctypes("/opt/axon/libaxon_pjrt.so")
    except Exception:
        pass


_ensure_ntff_hook()

B, S, H = 2, 2048, 2048
NH = 16
HD = 128
NHL = 4                      # heads per core
HL = NHL * HD                # 512 local head-dims
N_CORES = 8
F32 = mybir.dt.float32
BF16 = mybir.dt.bfloat16
SCALE = float(1.0 / np.sqrt(HD))

NHT = H // 128               # 16 contraction tiles
NQ = S // 128                # 16 query sub-tiles
NBLK = 4                     # 512-wide query blocks

LAST_RESULT = None
_CACHED_NC = None


def build_graph():
    import random
    random.seed(0)
    np.random.seed(0)
    nc = bacc.Bacc("TRN2", target_bir_lowering=False, num_devices=N_CORES)
    # x stays in [H, S] layout: per-tile DMAs read DRAM linearly.
    xT_d = nc.declare_dram_parameter("xT", [H, S], BF16, isOutput=False)
    # All weights are tile-ROW-major ([ntiles*128, cols]) so per-tile
    # slices are contiguous 64-512KB DRAM reads; they stream on the two
    # input queues in exact consumption order behind x.
    wq01_d = nc.declare_dram_parameter("wq01", [NHT * 128, 256], BF16,
                                       isOutput=False)
    wk01_d = nc.declare_dram_parameter("wk01", [NHT * 128, 256], BF16,
                                       isOutput=False)
    wq23_d = nc.declare_dram_parameter("wq23", [NHT * 128, 256], BF16,
                                       isOutput=False)
    wk23_d = nc.declare_dram_parameter("wk23", [NHT * 128, 256], BF16,
                                       isOutput=False)
    wv_d = nc.declare_dram_parameter("wvP", [NHT * 128, HL], BF16,
                                     isOutput=False)
    wp_d = nc.declare_dram_parameter("wpP", [NHL * 128, H], BF16,
                                     isOutput=False)
    onesb_d = nc.declare_dram_parameter("onesb", [128, 128], BF16,
                                        isOutput=False)
    # upper-triangular (incl diagonal) 0/1 keep-mask for post-exp zeroing
    m01_d = nc.declare_dram_parameter("m01", [128, 128], BF16, isOutput=False)
    out_d = nc.declare_dram_parameter("out", [S, H], BF16, isOutput=True)

    Copy = mybir.ActivationFunctionType.Copy
    Exp = mybir.ActivationFunctionType.Exp

    with tile.TileContext(nc) as tc:
        with tc.tile_pool(name="persist", bufs=1) as pp:
            # ---- persistent SBUF tiles ----
            xT = pp.tile([128, NHT * S], BF16, tag="x", name="xT")
            wq01 = pp.tile([128, NHT * 256], BF16, tag="wq01", name="wq01")
            wk01 = pp.tile([128, NHT * 256], BF16, tag="wk01", name="wk01")
            wq23 = pp.tile([128, NHT * 256], BF16, tag="wq23", name="wq23")
            wk23 = pp.tile([128, NHT * 256], BF16, tag="wk23", name="wk23")
            wv = pp.tile([128, NHT * HL], BF16, tag="wv", name="wv")
            wp = pp.tile([128, NHL * H], BF16, tag="wp", name="wp")
            qT = [pp.tile([128, S], BF16, tag=f"q{h}", name=f"qT{h}")
                  for h in range(NHL)]
            kT = [pp.tile([128, S], BF16, tag=f"k{h}", name=f"kT{h}")
                  for h in range(NHL)]
            v_sb = [pp.tile([128, HL], BF16, tag=f"v{t}", name=f"v{t}")
                    for t in range(NQ)]
            # yT double-buffered by block parity (block j in parity j%2)
            yT = [[pp.tile([128, 512], BF16, tag=f"y{h}_{par}",
                           name=f"y{h}_{par}") for par in range(2)]
                  for h in range(NHL)]
            onesb_sb = pp.tile([128, 128], BF16, tag="onesb", name="onesb_sb")
            m01_sb = pp.tile([128, 128], BF16, tag="m01", name="m01_sb")

            # ---- startup DMAs in pass-1 consumption order ----
            # tile h's weight slice + two x halves, alternating queues so
            # aggregate delivery stays in order and ahead of the PE.
            for h in range(NHT):
                eng = (nc.sync, nc.gpsimd, nc.scalar)[h % 3]
                eng.dma_start(wq01[:, 256 * h:256 * (h + 1)],
                              wq01_d[128 * h:128 * (h + 1), :])
                # finer chunks for the first tiles so the first matmul can
                # start as early as possible
                nx = 4 if h < 2 else 2
                cw = S // nx
                for ci in range(nx):
                    eng.dma_start(xT[:, S * h + cw * ci:S * h + cw * (ci + 1)],
                                  xT_d[128 * h:128 * (h + 1),
                                       cw * ci:cw * (ci + 1)])
            # Remaining weights stream per-tile on the same two queues right
            # behind x, in consumption order: wk01 slice h lands ~40us,
            # needed ~44+2h us (it previously arrived after pass 2 began).
            # A third queue would steal HBM bandwidth from the x stream.
            for wtile, wdram, cols in ((wk01, wk01_d, 256),
                                       (wq23, wq23_d, 256),
                                       (wk23, wk23_d, 256),
                                       (wv, wv_d, HL)):
                for h in range(NHT):
                    eng = nc.sync if h % 2 == 0 else nc.gpsimd
                    eng.dma_start(wtile[:, cols * h:cols * (h + 1)],
                                  wdram[128 * h:128 * (h + 1), :])
            for k in range(NHL):
                eng = nc.sync if k % 2 == 0 else nc.gpsimd
                eng.dma_start(wp[:, H * k:H * (k + 1)],
                              wp_d[128 * k:128 * (k + 1), :])
            nc.scalar.dma_start(onesb_sb[:], onesb_d[:])
            nc.scalar.dma_start(m01_sb[:], m01_d[:])

            # ---------------- Phase B: Q/K projections ----------------
            # Four passes (wq-hp01, wk-hp01, wq-hp23, wk-hp23); each pass
            # accumulates 8 PSUM tiles (2 heads x 4 token chunks) over the
            # 16 contraction tiles.  PSUM drains alternate Scalar/Vector.
            with tc.tile_pool(name="pB", bufs=8, space="PSUM") as pB:
                def proj_pass(wtile, dsts):
                    pss = [pB.tile([128, 512], F32, tag="pb", name="pb")
                           for _ in range(8)]
                    for h in range(NHT):
                        for gi in range(2):
                            lhsT = wtile[:, 256 * h + 128 * gi:
                                         256 * h + 128 * (gi + 1)]
                            for s in range(4):
                                nc.tensor.matmul(
                                    pss[4 * gi + s][:], lhsT,
                                    xT[:, S * h + 512 * s:
                                       S * h + 512 * (s + 1)],
                                    start=(h == 0), stop=(h == NHT - 1))
                    for gi in range(2):
                        for s in range(4):
                            dst = dsts[gi][:, 512 * s:512 * (s + 1)]
                            src = pss[4 * gi + s][:]
                            if s % 2 == 0:
                                nc.scalar.activation(dst, src, Copy)
                            else:
                                nc.vector.tensor_copy(dst, src)

                proj_pass(wq01, (qT[0], qT[1]))
                proj_pass(wk01, (kT[0], kT[1]))
                proj_pass(wq23, (qT[2], qT[3]))
                proj_pass(wk23, (kT[2], kT[3]))

            # ---------- Phase C: v-proj + attention + out-proj ----------
            with (
                tc.tile_pool(name="spp", bufs=2, space="PSUM") as spp,
                tc.tile_pool(name="ypp", bufs=2, space="PSUM") as ypp,
                tc.tile_pool(name="accp", bufs=2, space="PSUM") as accp,
                tc.tile_pool(name="ptp", bufs=5) as ptp,
                tc.tile_pool(name="bcp", bufs=1) as bcp,
                tc.tile_pool(name="otp", bufs=2) as otp,
                tc.tile_pool(name="lsp", bufs=2) as lsp,
            ):
                def vproj_mms(st, acc, h2s):
                    for h2 in h2s:
                        nc.tensor.matmul(
                            acc[:],
                            xT[:, S * h2 + 128 * st:S * h2 + 128 * (st + 1)],
                            wv[:, 512 * h2:512 * (h2 + 1)],
                            start=(h2 == 0), stop=(h2 == NHT - 1))

                def vproj_tile(st):
                    acc = accp.tile([128, 512], F32, tag="acc", name="vm")
                    vproj_mms(st, acc, range(NHT))
                    nc.scalar.activation(v_sb[st][:], acc[:], Copy)

                def oproj_chunk(q, o, ot):
                    jq, q4 = divmod(q, 4)
                    acc = accp.tile([128, 512], F32, tag="acc", name="po")
                    for k in range(4):
                        nc.tensor.matmul(
                            acc[:],
                            yT[k][jq % 2][:, 128 * q4:128 * (q4 + 1)],
                            wp[:, 2048 * k + 512 * o:
                               2048 * k + 512 * (o + 1)],
                            start=(k == 0), stop=(k == 3))
                    dst = ot[:, 512 * o:512 * (o + 1)]
                    # split drains across DVE and Scalar: DVE also carries
                    # the softmax-denominator chain and was the slot-end
                    # bottleneck
                    if o < 2:
                        nc.vector.tensor_copy(dst, acc[:])
                    else:
                        nc.scalar.activation(dst, acc[:], Copy)

                # block 0's own v-projection (token tiles 0..3)
                for i in range(4):
                    vproj_tile(i)

                for j in range(NBLK):
                    nt = 4 * (j + 1)
                    for h in range(NHL):
                        # All blocks: full k-tile PAIRS share one 2-bank
                        # PSUM tile so a single exp instruction covers 1024
                        # columns.  The softmax denominator is a bf16
                        # running sum of probs tiles on DVE, finished by ONE
                        # ones-matmul per head; masked diagonal probs are
                        # zeroed by a 0/1 mul on GpSimd.
                        yp = ypp.tile([128, 512], F32, tag="yt", name="yp")
                        lsum = lsp.tile([128, 512], BF16, tag="lsum",
                                        name="lsum")
                        pend = []
                        first_ls = True
                        npops = [0]

                        # Independent PE filler work for this slot, emitted
                        # one chunk at a time between score pairs and around
                        # the final PV drain: block j-1's out-projection and
                        # block j+1's v-projection.
                        fillers = []
                        if j > 0:
                            q = 4 * (j - 1) + h
                            ot = otp.tile([128, H], BF16, tag="ot", name="ot")

                            def mk_oproj(o, q=q, ot=ot):
                                def f():
                                    oproj_chunk(q, o, ot)
                                    if o == 3:
                                        nc.sync.dma_start(
                                            out_d[128 * q:128 * (q + 1), :],
                                            ot[:])
                                return f

                            for o in range(4):
                                fillers.append(mk_oproj(o))
                        if j < 3:
                            st = 4 * (j + 1) + h
                            vst = {}

                            def vh0(st=st, vst=vst):
                                vst["acc"] = accp.tile([128, 512], F32,
                                                       tag="acc", name="vm")
                                vproj_mms(st, vst["acc"], range(0, 8))

                            def vh1(st=st, vst=vst):
                                vproj_mms(st, vst["acc"], range(8, NHT))
                                nc.scalar.activation(v_sb[st][:],
                                                     vst["acc"][:], Copy)

                            fillers.append(vh0)
                            fillers.append(vh1)

                        def pv_pop():
                            t, tile_, base, r0 = pend.pop(0)
                            nc.tensor.matmul(
                                yp[:, r0:512],
                                v_sb[t][:, 128 * h:128 * (h + 1)],
                                tile_[:, base + r0:base + 512],
                                start=(npops[0] == 0),
                                stop=(npops[0] == nt - 1))
                            npops[0] += 1

                        # Diagonal singles FIRST: their exp -> mask -> pop
                        # chains span three engines, so issuing them early
                        # hides the latency under the dense full pairs, and
                        # the end-of-head drain only waits on short chains.
                        for t in range(4 * j, nt):
                            r0 = 0 if t <= 4 * j else 128 * (t - 4 * j)
                            spw = spp.tile([128, 1024], F32, tag="sc",
                                           name="sp")
                            ptw = ptp.tile([128, 1024], BF16, tag="pt",
                                           name="pt")
                            nc.tensor.matmul(
                                spw[:, r0:512],
                                kT[h][:, 128 * t:128 * (t + 1)],
                                qT[h][:, 512 * j + r0:512 * (j + 1)],
                                start=True, stop=True)
                            nc.scalar.activation(
                                ptw[:, r0:512], spw[:, r0:512], Exp,
                                scale=SCALE)
                            # zero the strictly-lower (masked) part of the
                            # diagonal 128-block
                            nc.gpsimd.tensor_mul(
                                ptw[:, r0:r0 + 128],
                                ptw[:, r0:r0 + 128], m01_sb[:])
                            if first_ls:
                                nc.vector.tensor_copy(lsum[:],
                                                      ptw[:, 0:512])
                                first_ls = False
                            else:
                                nc.vector.tensor_add(
                                    lsum[:, r0:512], lsum[:, r0:512],
                                    ptw[:, r0:512])
                            pend.append((t, ptw, 0, r0))
                            while len(pend) > 5:
                                pv_pop()
                        # full k-tile pairs; one 1024-col exp per pair.
                        # From the 3rd pair on, interleave one filler chunk
                        # per pair (keeping one in reserve for the final
                        # drain): the PE outruns exp by ~85ns/pair and would
                        # otherwise stall on the scores-PSUM ping-pong.
                        for idx, t in enumerate(range(0, 4 * j, 2)):
                            spw = spp.tile([128, 1024], F32, tag="sc",
                                           name="sp")
                            ptw = ptp.tile([128, 1024], BF16, tag="pt",
                                           name="pt")
                            for half in range(2):
                                nc.tensor.matmul(
                                    spw[:, 512 * half:512 * (half + 1)],
                                    kT[h][:, 128 * (t + half):
                                          128 * (t + half + 1)],
                                    qT[h][:, 512 * j:512 * (j + 1)],
                                    start=True, stop=True)
                            nc.scalar.activation(
                                ptw[:], spw[:], Exp, scale=SCALE)
                            nc.vector.tensor_add(lsum[:], lsum[:],
                                                 ptw[:, 0:512])
                            nc.vector.tensor_add(lsum[:], lsum[:],
                                                 ptw[:, 512:1024])
                            pend.append((t, ptw, 0, 0))
                            pend.append((t + 1, ptw, 512, 0))
                            while len(pend) > 5:
                                pv_pop()
                            if idx >= 2 and len(fillers) > 1:
                                fillers.pop(0)()

                        # one filler chunk while the last exp/mask ops drain
                        if fillers:
                            fillers.pop(0)()
                        while pend:
                            pv_pop()
                        # denominator chain immediately: recip + yT-mul go
                        # to the front of the DVE queue so the next slots'
                        # PV accumulator (PSUM WAR on yp) never waits
                        lp = spp.tile([128, 1024], F32, tag="sc", name="lp2")
                        nc.tensor.matmul(lp[:, 0:512], onesb_sb[:], lsum[:],
                                         start=True, stop=True)
                        bcs = bcp.tile([128, 512], F32, tag="bcs", name="bcs")
                        nc.vector.reciprocal_approx_fast(bcs[:], lp[:, 0:512])
                        nc.vector.tensor_mul(yT[h][j % 2][:], yp[:], bcs[:])
                        while fillers:
                            fillers.pop(0)()

            # tail: out-projection for the last block in its own PSUM pool
            # scope — the phase-C pools are closed, so four accumulator
            # banks are free and a chunk's drain never gates the next
            # chunk's matmuls
            with (
                tc.tile_pool(name="taccp", bufs=4, space="PSUM") as taccp,
                tc.tile_pool(name="totp", bufs=2) as totp,
            ):
                for q4 in range(4):
                    q = 12 + q4
                    ot = totp.tile([128, H], BF16, tag="ot", name="ott")
                    for o in range(4):
                        acc = taccp.tile([128, 512], F32, tag="acc",
                                         name="pot")
                        for k in range(4):
                            nc.tensor.matmul(
                                acc[:],
                                yT[k][1][:, 128 * q4:128 * (q4 + 1)],
                                wp[:, 2048 * k + 512 * o:
                                   2048 * k + 512 * (o + 1)],
                                start=(k == 0), stop=(k == 3))
                        dst = ot[:, 512 * o:512 * (o + 1)]
                        if o % 2 == 1:
                            nc.scalar.activation(dst, acc[:], Copy)
                        else:
                            nc.vector.tensor_copy(dst, acc[:])
                        # store per chunk, fanned across queues, so only the
                        # last 512 columns remain in flight at run end
                        eng = (nc.sync, nc.gpsimd, nc.scalar, nc.sync)[o]
                        eng.dma_start(
                            out_d[128 * q:128 * (q + 1),
                                  512 * o:512 * (o + 1)], dst)

    nc.compile()
    return nc


def _get_nc():
    global _CACHED_NC
    if _CACHED_NC is None:
        _CACHED_NC = build_graph()
    return _CACHED_NC


def kernel(x, w_attn, w_proj):
    global LAST_RESULT
    nc = _get_nc()
    onesb = np.ones((128, 128), bfloat16)
    m01 = np.triu(np.ones((128, 128), np.float32)).astype(bfloat16)
    in_maps = []
    for c in range(N_CORES):
        b, g = divmod(c, 4)
        lo, hi = HL * g, HL * (g + 1)
        xTb = np.ascontiguousarray(x[b].T).astype(bfloat16)
        wqT = np.ascontiguousarray(w_attn[lo:hi, :].T).astype(bfloat16)
        wkT = np.ascontiguousarray(w_attn[H + lo:H + hi, :].T).astype(bfloat16)
        wvT = np.ascontiguousarray(w_attn[2 * H + lo:2 * H + hi, :].T
                                   ).astype(bfloat16)
        wpT = np.ascontiguousarray(w_proj[:, lo:hi].T).astype(bfloat16)
        in_maps.append({
            "xT": xTb,
            # weights tile-row-major: rows 128h..128h+128 = contraction
            # tile h (contiguous DRAM block per slice DMA)
            "wq01": np.ascontiguousarray(wqT[:, 0:256]),
            "wk01": np.ascontiguousarray(wkT[:, 0:256]),
            "wq23": np.ascontiguousarray(wqT[:, 256:512]),
            "wk23": np.ascontiguousarray(wkT[:, 256:512]),
            "wvP": wvT,
            "wpP": wpT,
            "onesb": onesb,
            "m01": m01,
        })
    res = run_bass_kernel_spmd(nc, in_maps, core_ids=list(range(N_CORES)))
    LAST_RESULT = res
    outs = [np.asarray(res.results[c]["out"], dtype=np.float32)
            for c in range(N_CORES)]
    out = np.empty((B, S, H), np.float32)
    out[0] = outs[0] + outs[1] + outs[2] + outs[3]
    out[1] = outs[4] + outs[5] + outs[6] + outs[7]
    return out
